# revision 30
# baseline (speedup 1.0000x reference)
"""Contrastive loss (InfoNCE, mean reduction) on 8 Trainium2 NeuronCores.

Reference computation (B=16384, D=64, fp32):
    a = embeddings_a / ||embeddings_a||_row ; b likewise
    sim = a @ b.T / 0.07
    loss = mean_i( logsumexp(sim[i, :]) - sim[i, i] )

Sharding: rows of `a` split across 8 cores (2048 rows each); every core gets
the full `b`. Each core computes its [2048, 16384] block of sim on the fly in
PSUM (never hitting HBM), exp + row-sum fused on ScalarE (optionally partially
offloaded to VectorE via a Schraudolph-style exp), then log - diag. The host
sums the 8x2048 per-row losses and divides by B.

Numerics: sim values lie in [-1/0.07, 1/0.07] ~= [-14.3, 14.3]; exp never
overflows fp32, so logsumexp's max-subtraction is skipped.

Layout notes:
 - matmul contracts over partitions, so both operands live transposed as
   [D, rows] in bf16. Transposes are done by the DMA XBAR (needs free%128==0,
   hence b/a tiles are padded to 128 columns with zeros).
 - tensor_tensor_reduce hard-crashes this HW/runtime; scalar_tensor_tensor
   or tensor_mul+tensor_reduce are used instead.
"""

import sys

sys.path.insert(0, "/opt/trn_rl_repo")

import numpy as np

B = 16384
D = 64
TEMP = 0.07
NCORES = 8
RPC = B // NCORES  # rows per core = 2048
NT_A = RPC // 128  # a tiles per core = 16
NT_B = B // 128  # b tiles = 128
JC = 2048  # sim columns per psum tile (4 banks)
NJ = B // JC  # j chunks = 8
TPC = JC // 128  # b tiles per chunk = 16

# VectorE exp offload (Schraudolph bit-trick): which j-chunks are computed on
# VectorE instead of ScalarE. Empty set = all on ScalarE.
# Offload every OFFLOAD_MOD-th (it, jc) cell's exp+rowsum to VectorE
# (0 = disabled). Interleaving cells (not whole chunks) keeps the PSUM slot
# rotation feeding both ScalarE and VectorE concurrently.
OFFLOAD_MOD = 0  # measured: any VectorE exp offload slows the 2-slot PSUM
                 # pipeline (cell latency gates, not engine throughput)
OFFLOAD_JC = ()  # legacy, unused
# exp(x) ~= bitcast_f32(int32(x * 2^23/ln2 + (127*2^23 - C)))
# C calibrated numerically for zero sum-bias on x ~ N(0, 1.8) (sim values):
# bias 2.3e-6, max per-element rel err 3.9% (averages out over 2048-col sums).
SCHRAUDOLPH_C = 483000.0
SCHRAUDOLPH_CORR = 1.0  # multiplicative bias correction for offloaded sums

_CACHE = {}


def _build_program(mm_dtype="bfloat16", offload_jc=None, offload_mod=None,
                   debug_outs=False):
    from contextlib import ExitStack

    import concourse.bacc as bacc
    import concourse.tile as tile
    from concourse import mybir

    if offload_jc is None:
        offload_jc = OFFLOAD_JC
    offload_jc = set(offload_jc)
    if offload_mod is None:
        offload_mod = OFFLOAD_MOD

    f32 = mybir.dt.float32
    i32 = mybir.dt.int32
    AF = mybir.ActivationFunctionType
    OP = mybir.AluOpType
    mm_dt = getattr(mybir.dt, mm_dtype)

    nc = bacc.Bacc("TRN2", target_bir_lowering=False, debug=False)
    a_ap = nc.dram_tensor("a", [RPC, D], f32, kind="ExternalInput").ap()
    b_ap = nc.dram_tensor("b", [B, D], f32, kind="ExternalInput").ap()
    bd_ap = nc.dram_tensor("bdiag", [RPC, D], f32, kind="ExternalInput").ap()
    out_ap = nc.dram_tensor("losses", [128, NT_A], f32, kind="ExternalOutput").ap()
    if debug_outs:
        dbg_rsp = nc.dram_tensor("dbg_rsp", [128, NT_A, NJ], f32, kind="ExternalOutput").ap()

    with ExitStack() as ctx:
        tc = ctx.enter_context(tile.TileContext(nc))
        big = ctx.enter_context(tc.tile_pool(name="big", bufs=1))
        prep = ctx.enter_context(tc.tile_pool(name="prep", bufs=4))
        stats = ctx.enter_context(tc.tile_pool(name="stats", bufs=1))

        # --- load inputs; b padded to 128 cols (zeros in 64:128) ---
        b_nat = big.tile([128, NT_B, 128], f32, tag="b_nat")
        nc.vector.memset(b_nat[:, :, D:], 0)
        b_r = b_ap.rearrange("(t p) d -> p t d", p=128)
        for g in range(NJ):
            nc.sync.dma_start(
                b_nat[:, g * TPC : (g + 1) * TPC, :D], b_r[:, g * TPC : (g + 1) * TPC, :]
            )
        a_nat = big.tile([128, NT_A, 128], f32, tag="a_nat")
        nc.vector.memset(a_nat[:, :, D:], 0)
        nc.sync.dma_start(a_nat[:, :, :D], a_ap.rearrange("(t p) d -> p t d", p=128))
        bd_nat = big.tile([128, NT_A, D], f32, tag="bd_nat")
        nc.sync.dma_start(bd_nat[:], bd_ap.rearrange("(t p) d -> p t d", p=128))

        # --- row norms (batched on VectorE), rsqrt via ACT Sqrt + DVE recip ---
        def norms_sq(src3d, n_tiles, tag, ncols=D):
            nsq = stats.tile([128, n_tiles], f32, tag=f"nsq_{tag}")
            step = min(16, n_tiles)
            for g in range(0, n_tiles, step):
                scr = prep.tile([128, step, ncols], f32, tag=f"scr_{ncols}")
                nc.vector.tensor_mul(
                    scr[:], src3d[:, g : g + step, :ncols], src3d[:, g : g + step, :ncols]
                )
                nc.vector.tensor_reduce(
                    nsq[:, g : g + step], scr[:], axis=mybir.AxisListType.X, op=OP.add
                )
            return nsq

        # Split the b-norm sqrt: chunk 0's columns first (unblocks chunk-0
        # prep ~20us earlier), remaining columns in a second op — both run
        # before the first Exp, so the ACT table set switches only once.
        nsq_b = norms_sq(b_nat, NT_B, "b")
        rb = stats.tile([128, NT_B], f32, tag="rb")
        nc.scalar.activation(rb[:, :TPC], nsq_b[:, :TPC], AF.Sqrt)
        nc.vector.reciprocal(rb[:, :TPC], rb[:, :TPC])
        nc.scalar.activation(rb[:, TPC:], nsq_b[:, TPC:], AF.Sqrt)
        nc.vector.reciprocal(rb[:, TPC:], rb[:, TPC:])

        nsq_a = norms_sq(a_nat, NT_A, "a")
        ra = stats.tile([128, NT_A], f32, tag="ra")
        nc.scalar.activation(ra[:], nsq_a[:], AF.Sqrt, scale=TEMP * TEMP)
        nc.vector.reciprocal(ra[:], ra[:])

        nsq_bd = norms_sq(bd_nat, NT_A, "bd")
        rbd = stats.tile([128, NT_A], f32, tag="rbd")
        nc.scalar.activation(rbd[:], nsq_bd[:], AF.Sqrt)
        nc.vector.reciprocal(rbd[:], rbd[:])

        # --- diag_i = (a_i . b_i) * ra_i * rbd_i == sim[i, i] ---
        diag = stats.tile([128, NT_A], f32, tag="diag")
        scr_d = prep.tile([128, NT_A, D], f32, tag="scr_64")
        nc.vector.tensor_mul(scr_d[:], a_nat[:, :, :D], bd_nat[:])
        nc.vector.tensor_reduce(diag[:], scr_d[:], axis=mybir.AxisListType.X, op=OP.add)
        nc.vector.tensor_mul(diag[:], diag[:], ra[:])
        nc.vector.tensor_mul(diag[:], diag[:], rbd[:])

        # --- transposed scaled operands via scale-cast + XBAR dma transpose ---
        # XBAR transposes serialize per HW queue (~1.2us each); alternate the
        # two HWDGE issuers (sync, scalar) to run two queues in parallel.
        xbar_eng = [nc.sync, nc.scalar]

        aT = big.tile([128, RPC], mm_dt, tag="aT")  # partitions 0:64 hold data
        for t in range(NT_A):
            asx = prep.tile([128, 128], mm_dt, tag="sc")
            nc.vector.tensor_scalar_mul(asx[:], a_nat[:, t, :], ra[:, t : t + 1])
            xbar_eng[t % 2].dma_start_transpose(aT[:, t * 128 : (t + 1) * 128], asx[:])

        bT = big.tile([128, B], mm_dt, tag="bT")
        rs_parts = stats.tile([128, NT_A, NJ], f32, tag="rsp")
        mpsum = ctx.enter_context(tc.tile_pool(name="mpsum", bufs=2, space="PSUM"))

        S1 = float(2.0**23 / np.log(2.0))
        S2 = float(127.0 * 2.0**23 - SCHRAUDOLPH_C)

        for jc in range(NJ):
            # prep this chunk's bT columns
            for t in range(jc * TPC, (jc + 1) * TPC):
                bs = prep.tile([128, 128], mm_dt, tag="sc")
                nc.vector.tensor_scalar_mul(bs[:], b_nat[:, t, :], rb[:, t : t + 1])
                eng = xbar_eng[t % 2] if jc == 0 else nc.sync
                eng.dma_start_transpose(bT[:, t * 128 : (t + 1) * 128], bs[:])
            # main: all a tiles against this chunk
            for it in range(NT_A):
                lhs = aT[:64, it * 128 : (it + 1) * 128]
                ps = mpsum.tile([128, JC], f32, tag="ps")
                for k in range(JC // 512):
                    col = jc * JC + k * 512
                    nc.tensor.matmul(
                        ps[:, k * 512 : (k + 1) * 512],
                        lhsT=lhs,
                        rhs=bT[:64, col : col + 512],
                        start=True,
                        stop=True,
                    )
                off = jc in offload_jc or (
                    offload_mod and (it + jc) % offload_mod == 0
                )
                if off:
                    # Schraudolph exp + reduce on VectorE
                    ex = prep.tile([128, JC], i32, tag="ex")
                    nc.vector.tensor_scalar(
                        ex[:], ps[:], S1, S2, op0=OP.mult, op1=OP.add
                    )
                    nc.vector.tensor_reduce(
                        rs_parts[:, it, jc : jc + 1],
                        ex[:].bitcast(f32),
                        axis=mybir.AxisListType.X,
                        op=OP.add,
                    )
                else:
                    nc.scalar.activation(
                        ps[:], ps[:], AF.Exp, accum_out=rs_parts[:, it, jc : jc + 1]
                    )

        if debug_outs:
            nc.sync.dma_start(dbg_rsp[:], rs_parts[:])

        # --- lse = ln(sum of parts); loss = lse - diag ---
        if offload_jc and SCHRAUDOLPH_CORR != 1.0:
            for jc in sorted(offload_jc):
                nc.vector.tensor_scalar_mul(
                    rs_parts[:, :, jc : jc + 1], rs_parts[:, :, jc : jc + 1],
                    SCHRAUDOLPH_CORR,
                )
        rowsum = stats.tile([128, NT_A], f32, tag="rowsum")
        nc.vector.tensor_reduce(
            rowsum[:], rs_parts[:], axis=mybir.AxisListType.X, op=OP.add
        )
        lse = stats.tile([128, NT_A], f32, tag="lse")
        nc.scalar.activation(lse[:], rowsum[:], AF.Ln)
        out_sb = stats.tile([128, NT_A], f32, tag="out_sb")
        nc.vector.tensor_sub(out_sb[:], lse[:], diag[:])
        nc.sync.dma_start(out_ap[:], out_sb[:])

    nc.compile()
    return nc


def get_program():
    if "nc" not in _CACHE:
        _CACHE["nc"] = _build_program()
    return _CACHE["nc"]


def make_in_maps(a, b):
    return [
        {
            "a": np.ascontiguousarray(a[c * RPC : (c + 1) * RPC]),
            "b": b,
            "bdiag": np.ascontiguousarray(b[c * RPC : (c + 1) * RPC]),
        }
        for c in range(NCORES)
    ]


def kernel(embeddings_a, embeddings_b):
    from concourse.bass_utils import run_bass_kernel_spmd

    a = np.ascontiguousarray(np.asarray(embeddings_a, dtype=np.float32))
    b = np.ascontiguousarray(np.asarray(embeddings_b, dtype=np.float32))
    assert a.shape == (B, D) and b.shape == (B, D)

    nc = get_program()
    res = run_bass_kernel_spmd(nc, make_in_maps(a, b), core_ids=list(range(NCORES)))
    total = 0.0
    for c in range(NCORES):
        total += res.results[c]["losses"].astype(np.float64).sum()
    return np.float32(total / B)
